# revision 41
# baseline (speedup 1.0000x reference)
"""Trainium2 Bass kernel for the interval-prediction custom loss.

total = 10*mean((t - (l+u)/2)^2) + 0.1*mean(u-l) + 10*mean(relu(l-u))
        + 0.5*sum(where(pv==0, relu(c-p), relu(p-c)))/N        with c=(l+u)/2

Strategy: pure data parallel over N across 8 NeuronCores; host does only the
tiny final scalar reduction in float64.

Engine plan (v12).  Measured facts from earlier traces: DVE tensor_tensor
runs 2x and tensor_scalar 4x at bf16, but ANY DVE op with an accumulator
drops to 1x; GPSIMD elementwise ops steal the DVE's SBUF port (2x
slowdown); heavy PE matmul traffic slows the DMA streams (SBUF bandwidth
contention); ACT costs ~(fd+352)/1.2 ns per pass regardless of dtype.  So:
DVE does all seven elementwise tiles in fast modes, ACT does the two
nonlinear accumulations, PE does light ones-matmul column sums (one
stationary, loaded once):

  DVE:    H = lo + up            (TT 2x)
          w = lo - up            (TT 2x)
          c = 0.5*H              (TS 4x)
          x = c - p              (TT 2x)
          q = v * x              (TT 2x)
          rxt = max(x, 0)        (TS 4x)
          e = c - t              (TT 2x, last: its consumer can lag)
  ACT:    Relu(w)    accum -> S_vd    (= sum relu(lo-up))
          Square(e)  accum -> S_sq
  PE:     ones^T * w   -> PSUM[1,512]   (S_w;  width sum = -S_w)
          ones^T * rxt -> PSUM[1,512]   (S_rx)
          ones^T * q   -> PSUM[1,512]   (S_vx)

Host combine: width = -S_w/N, direction = S_rx - S_vx.

All five streams are bf16 (int64 pv is 0/1 so the cast is exact); all
accumulation is fp32.  Tile widths are multiples of 512 so each PE matmul
chunk is exactly 512 (one PSUM bank).  [lo,up] is a separate SBUF tile from
[t,p,v] so H/w start as soon as the first DMA lands; io pools are deep
enough that no dma_start ever waits on a tile free (head-of-line blocking
on the sync sequencer stalls all 16 DMA queues).
"""

import sys

if "/opt/trn_rl_repo" not in sys.path:
    sys.path.insert(0, "/opt/trn_rl_repo")

import numpy as np

N = 8388608
N_CORES = 8
P = 128
NP_PER_CORE = N // N_CORES            # 1048576
FPL = NP_PER_CORE // P                # 8192 elements per partition lane
TILE_WIDTHS = (512, 2048, 2048, 2048, 1024, 512)
assert sum(TILE_WIDTHS) == FPL
assert all(w % 512 == 0 for w in TILE_WIDTHS)
MM = 512                              # matmul moving chunk / PSUM bank width

_NC_CACHE = {}


def _build(fpl=FPL, widths=TILE_WIDTHS):
    """Build the per-core Bass program (identical on all cores)."""
    from concourse import bacc, mybir
    from concourse.tile import TileContext

    assert sum(widths) == fpl
    n_tiles = len(widths)

    f32 = mybir.dt.float32
    bf16 = mybir.dt.bfloat16
    Alu = mybir.AluOpType
    Act = mybir.ActivationFunctionType

    nc = bacc.Bacc(trn_type="TRN2")
    big = nc.declare_dram_parameter("big", [P, 5 * fpl], bf16, isOutput=False)
    consts = nc.declare_dram_parameter("consts", [P, 1], bf16, isOutput=False)
    # accumulator columns: S_sq (n_tiles) | S_vd (n_tiles)
    out = nc.declare_dram_parameter("out", [P, 2 * n_tiles], f32, isOutput=True)
    # rows: S_w | S_rx | S_vx column sums
    sums = nc.declare_dram_parameter("sums", [3, MM], f32, isOutput=True)

    with TileContext(nc) as tc:
        with (
            tc.tile_pool(name="ioa", bufs=5) as ioa_pool,
            tc.tile_pool(name="iob", bufs=5) as iob_pool,
            tc.tile_pool(name="mid", bufs=3) as mid_pool,
            tc.tile_pool(name="jnk", bufs=2) as jnk_pool,
            tc.tile_pool(name="acc", bufs=1) as acc_pool,
            tc.tile_pool(name="pss", bufs=1, space="PSUM") as pss_pool,
        ):
            acc_act = acc_pool.tile([P, 2 * n_tiles], f32, tag="acc_act")
            sw_sb = acc_pool.tile([1, MM], f32, tag="sw_sb")
            srx_sb = acc_pool.tile([1, MM], f32, tag="srx_sb")
            svx_sb = acc_pool.tile([1, MM], f32, tag="svx_sb")
            ps_w = pss_pool.tile([1, MM], f32, tag="ps_w")
            ps_rx = pss_pool.tile([1, MM], f32, tag="ps_rx")
            ps_vx = pss_pool.tile([1, MM], f32, tag="ps_vx")

            const_t = acc_pool.tile([P, 1], bf16, tag="consts")
            onesv = const_t[:, 0:1]

            off = 0
            first = [True, True, True]
            n_chunks = fpl // MM
            done_chunks = 0
            for j, tw in enumerate(widths):
                big_a = ioa_pool.tile([P, 2, tw], bf16, tag="biga", name=f"biga{j}")
                big_b = iob_pool.tile([P, 3, tw], bf16, tag="bigb", name=f"bigb{j}")
                src = big[:, off : off + 5 * tw].rearrange("p (s f) -> p s f", s=5)
                nc.sync.dma_start(out=big_a, in_=src[:, 0:2, :])
                nc.sync.dma_start(out=big_b, in_=src[:, 2:5, :])
                if j == 0:
                    # consts needed only by the PE; issue after the first
                    # tile's compute-critical DMAs
                    nc.sync.dma_start(out=const_t, in_=consts[:, :])
                off += 5 * tw

                lo = big_a[:, 0, :]
                up = big_a[:, 1, :]
                t_t = big_b[:, 0, :]
                p_t = big_b[:, 1, :]
                v_t = big_b[:, 2, :]

                H = mid_pool.tile([P, tw], bf16, tag="H", name=f"H{j}")
                w = mid_pool.tile([P, tw], bf16, tag="w", name=f"w{j}")
                c = mid_pool.tile([P, tw], bf16, tag="c", name=f"c{j}")
                x = mid_pool.tile([P, tw], bf16, tag="x", name=f"x{j}")
                e = mid_pool.tile([P, tw], bf16, tag="e", name=f"e{j}")
                q = mid_pool.tile([P, tw], bf16, tag="q", name=f"q{j}")
                rxt = mid_pool.tile([P, tw], bf16, tag="rxt", name=f"rxt{j}")
                ja = jnk_pool.tile([P, tw], bf16, tag="ja", name=f"ja{j}")
                jd = jnk_pool.tile([P, tw], bf16, tag="jd", name=f"jd{j}")

                # --- DVE: fast-mode ops only, no accumulators ---
                nc.vector.tensor_add(out=H, in0=lo, in1=up)
                nc.vector.tensor_sub(out=w, in0=lo, in1=up)
                nc.vector.tensor_scalar(
                    out=c, in0=H, scalar1=0.5, scalar2=None, op0=Alu.mult
                )
                nc.vector.tensor_sub(out=x, in0=c, in1=p_t)
                nc.vector.tensor_mul(out=q, in0=v_t, in1=x)
                nc.vector.tensor_scalar(
                    out=rxt, in0=x, scalar1=0.0, scalar2=None, op0=Alu.max
                )
                nc.vector.tensor_sub(out=e, in0=c, in1=t_t)

                # --- ACT: the two nonlinear accumulations ---
                nc.scalar.activation(
                    out=jd, in_=w, func=Act.Relu,
                    accum_out=acc_act[:, n_tiles + j : n_tiles + j + 1],
                )
                nc.scalar.activation(
                    out=ja, in_=e, func=Act.Square,
                    accum_out=acc_act[:, j : j + 1],
                )

                # --- PE: column-sum matmuls (single ones stationary) ---
                for ci, ch in enumerate(range(0, tw, MM)):
                    is_last = done_chunks + ci == n_chunks - 1
                    nc.tensor.matmul(
                        ps_w, onesv, w[:, ch : ch + MM],
                        start=first[0], stop=is_last,
                    )
                    first[0] = False
                for ci, ch in enumerate(range(0, tw, MM)):
                    is_last = done_chunks + ci == n_chunks - 1
                    nc.tensor.matmul(
                        ps_rx, onesv, rxt[:, ch : ch + MM],
                        start=first[1], stop=is_last,
                    )
                    first[1] = False
                for ci, ch in enumerate(range(0, tw, MM)):
                    is_last = done_chunks + ci == n_chunks - 1
                    nc.tensor.matmul(
                        ps_vx, onesv, q[:, ch : ch + MM],
                        start=first[2], stop=is_last,
                    )
                    first[2] = False
                done_chunks += tw // MM

            # PSUM -> SBUF -> DRAM for the column sums
            nc.scalar.activation(out=sw_sb[:, :], in_=ps_w, func=Act.Copy)
            nc.scalar.activation(out=srx_sb[:, :], in_=ps_rx, func=Act.Copy)
            nc.scalar.activation(out=svx_sb[:, :], in_=ps_vx, func=Act.Copy)

            nc.sync.dma_start(out=out[:, :], in_=acc_act)
            nc.sync.dma_start(out=sums[0:1, :], in_=sw_sb)
            nc.sync.dma_start(out=sums[1:2, :], in_=srx_sb)
            nc.sync.dma_start(out=sums[2:3, :], in_=svx_sb)

    nc.compile()
    return nc


def _get_nc():
    key = (FPL, TILE_WIDTHS)
    if key not in _NC_CACHE:
        _NC_CACHE[key] = _build()
    return _NC_CACHE[key]


def _make_consts():
    import ml_dtypes

    return np.ones((P, 1), dtype=ml_dtypes.bfloat16)


def _shard(inputs, fpl=FPL, widths=TILE_WIDTHS, n_cores=N_CORES):
    import ml_dtypes

    bf = ml_dtypes.bfloat16
    n = n_cores * P * fpl
    pred = np.asarray(inputs["pred"])
    targ = np.asarray(inputs["target"]).reshape(n)
    prev = np.asarray(inputs["prev_pci"]).reshape(n)
    # int64 is unsupported on-device; values are 0/1 so a bf16 cast is exact.
    pv = np.asarray(inputs["pv_values"]).astype(bf).reshape(n)

    lo = pred[:, 0].astype(bf)
    up = pred[:, 1].astype(bf)
    tb = targ.astype(bf)
    pb = prev.astype(bf)

    consts = _make_consts()
    np_per_core = P * fpl

    in_maps = []
    for cix in range(n_cores):
        s = slice(cix * np_per_core, (cix + 1) * np_per_core)
        streams = (
            lo[s].reshape(P, fpl),
            up[s].reshape(P, fpl),
            tb[s].reshape(P, fpl),
            pb[s].reshape(P, fpl),
            pv[s].reshape(P, fpl),
        )
        # tile-major: per partition, each tile's 5 stream-chunks contiguous
        parts = []
        off = 0
        for fd in widths:
            for st in streams:
                parts.append(st[:, off : off + fd])
            off += fd
        big = np.concatenate(parts, axis=1)
        in_maps.append({"big": np.ascontiguousarray(big), "consts": consts})
    return in_maps


def _combine(core_outs, core_sums, widths=TILE_WIDTHS, n=N):
    """core_outs: [P, 2*n_tiles] ACT accumulators per core (S_sq | S_vd).
    core_sums: [3, MM] column sums per core (S_w | S_rx | S_vx)."""
    n_tiles = len(widths)
    s_sq = s_vd = s_w = s_rx = s_vx = 0.0
    for o, ss in zip(core_outs, core_sums):
        o = np.asarray(o, dtype=np.float64)
        ss = np.asarray(ss, dtype=np.float64)
        s_sq += o[:, 0:n_tiles].sum()
        s_vd += o[:, n_tiles : 2 * n_tiles].sum()
        s_w += ss[0].sum()
        s_rx += ss[1].sum()
        s_vx += ss[2].sum()

    center_loss = s_sq / n
    width_loss = -s_w / n                  # sum(up - lo) = -sum(lo - up)
    valid_penalty = s_vd / n
    direction_penalty = s_rx - s_vx
    total = (
        center_loss * 10.0
        + 0.1 * width_loss
        + 10.0 * valid_penalty
        + 0.5 * direction_penalty / n
    )
    return np.float32(total)


def _run(inputs, trace=False):
    """Run the SPMD kernel; returns (scalar_result, BassKernelResults)."""
    from concourse.bass_utils import run_bass_kernel_spmd

    nc = _get_nc()
    in_maps = _shard(inputs)
    res = run_bass_kernel_spmd(
        nc, in_maps, core_ids=list(range(N_CORES)), trace=trace
    )
    core_outs = [res.results[c]["out"] for c in range(N_CORES)]
    core_sums = [res.results[c]["sums"] for c in range(N_CORES)]
    return _combine(core_outs, core_sums), res


def kernel(**inputs) -> np.ndarray:
    result, _ = _run(inputs, trace=False)
    return result


# revision 43
# speedup vs baseline: 1.0150x; 1.0150x over previous
"""Trainium2 Bass kernel for the interval-prediction custom loss.

total = 10*mean((t - (l+u)/2)^2) + 0.1*mean(u-l) + 10*mean(relu(l-u))
        + 0.5*sum(where(pv==0, relu(c-p), relu(p-c)))/N        with c=(l+u)/2

Strategy: pure data parallel over N across 8 NeuronCores; host does only the
tiny final scalar reduction in float64.

Engine plan (v12).  Measured facts from earlier traces: DVE tensor_tensor
runs 2x and tensor_scalar 4x at bf16, but ANY DVE op with an accumulator
drops to 1x; GPSIMD elementwise ops steal the DVE's SBUF port (2x
slowdown); heavy PE matmul traffic slows the DMA streams (SBUF bandwidth
contention); ACT costs ~(fd+352)/1.2 ns per pass regardless of dtype.  So:
DVE does all seven elementwise tiles in fast modes, ACT does the two
nonlinear accumulations, PE does light ones-matmul column sums (one
stationary, loaded once):

  DVE:    H = lo + up            (TT 2x)
          w = lo - up            (TT 2x)
          c = 0.5*H              (TS 4x)
          x = c - p              (TT 2x)
          q = v * x              (TT 2x)
          rxt = max(x, 0)        (TS 4x)
          e = c - t              (TT 2x, last: its consumer can lag)
  ACT:    Relu(w)    accum -> S_vd    (= sum relu(lo-up))
          Square(e)  accum -> S_sq
  PE:     ones^T * w   -> PSUM[1,512]   (S_w;  width sum = -S_w)
          ones^T * rxt -> PSUM[1,512]   (S_rx)
          ones^T * q   -> PSUM[1,512]   (S_vx)

Host combine: width = -S_w/N, direction = S_rx - S_vx.

All five streams are bf16 (int64 pv is 0/1 so the cast is exact); all
accumulation is fp32.  Tile widths are multiples of 512 so each PE matmul
chunk is exactly 512 (one PSUM bank).  [lo,up] is a separate SBUF tile from
[t,p,v] so H/w start as soon as the first DMA lands; io pools are deep
enough that no dma_start ever waits on a tile free (head-of-line blocking
on the sync sequencer stalls all 16 DMA queues).
"""

import sys

if "/opt/trn_rl_repo" not in sys.path:
    sys.path.insert(0, "/opt/trn_rl_repo")

import numpy as np

N = 8388608
N_CORES = 8
P = 128
NP_PER_CORE = N // N_CORES            # 1048576
FPL = NP_PER_CORE // P                # 8192 elements per partition lane
TILE_WIDTHS = (512, 2048, 2048, 2048, 1024, 512)
assert sum(TILE_WIDTHS) == FPL
assert all(w % 512 == 0 for w in TILE_WIDTHS)
MM = 512                              # matmul moving chunk / PSUM bank width

_NC_CACHE = {}


def _build(fpl=FPL, widths=TILE_WIDTHS):
    """Build the per-core Bass program (identical on all cores)."""
    from concourse import bacc, mybir
    from concourse.tile import TileContext

    assert sum(widths) == fpl
    n_tiles = len(widths)

    f32 = mybir.dt.float32
    bf16 = mybir.dt.bfloat16
    Alu = mybir.AluOpType
    Act = mybir.ActivationFunctionType

    nc = bacc.Bacc(trn_type="TRN2")
    big = nc.declare_dram_parameter("big", [P, 5 * fpl], bf16, isOutput=False)
    consts = nc.declare_dram_parameter("consts", [P, 1], bf16, isOutput=False)
    # accumulator columns: S_sq (n_tiles) | S_vd (n_tiles)
    out = nc.declare_dram_parameter("out", [P, 2 * n_tiles], f32, isOutput=True)
    # rows: S_w | S_rx | S_vx column sums
    sums = nc.declare_dram_parameter("sums", [3, MM], f32, isOutput=True)

    with TileContext(nc) as tc:
        with (
            tc.tile_pool(name="ioa", bufs=5) as ioa_pool,
            tc.tile_pool(name="iob", bufs=5) as iob_pool,
            tc.tile_pool(name="mid", bufs=3) as mid_pool,
            tc.tile_pool(name="jnk", bufs=2) as jnk_pool,
            tc.tile_pool(name="acc", bufs=1) as acc_pool,
            tc.tile_pool(name="pss", bufs=1, space="PSUM") as pss_pool,
        ):
            acc_act = acc_pool.tile([P, 2 * n_tiles], f32, tag="acc_act")
            sw_sb = acc_pool.tile([1, MM], f32, tag="sw_sb")
            srx_sb = acc_pool.tile([1, MM], f32, tag="srx_sb")
            svx_sb = acc_pool.tile([1, MM], f32, tag="svx_sb")
            ps_w = pss_pool.tile([1, MM], f32, tag="ps_w")
            ps_rx = pss_pool.tile([1, MM], f32, tag="ps_rx")
            ps_vx = pss_pool.tile([1, MM], f32, tag="ps_vx")

            const_t = acc_pool.tile([P, 1], bf16, tag="consts")
            onesv = const_t[:, 0:1]

            off = 0
            first = [True, True, True]
            n_chunks = fpl // MM
            done_chunks = 0
            for j, tw in enumerate(widths):
                big_a = ioa_pool.tile([P, 2, tw], bf16, tag="biga", name=f"biga{j}")
                big_b = iob_pool.tile([P, 3, tw], bf16, tag="bigb", name=f"bigb{j}")
                src = big[:, off : off + 5 * tw].rearrange("p (s f) -> p s f", s=5)
                nc.sync.dma_start(out=big_a, in_=src[:, 0:2, :])
                if j == 0:
                    # consts needed only by the PE; issue after the first
                    # compute-critical DMA
                    nc.sync.dma_start(out=const_t, in_=consts[:, :])
                nc.sync.dma_start(out=big_b, in_=src[:, 2:5, :])
                off += 5 * tw

                lo = big_a[:, 0, :]
                up = big_a[:, 1, :]
                t_t = big_b[:, 0, :]
                p_t = big_b[:, 1, :]
                v_t = big_b[:, 2, :]

                H = mid_pool.tile([P, tw], bf16, tag="H", name=f"H{j}")
                w = mid_pool.tile([P, tw], bf16, tag="w", name=f"w{j}")
                c = mid_pool.tile([P, tw], bf16, tag="c", name=f"c{j}")
                x = mid_pool.tile([P, tw], bf16, tag="x", name=f"x{j}")
                e = mid_pool.tile([P, tw], bf16, tag="e", name=f"e{j}")
                q = mid_pool.tile([P, tw], bf16, tag="q", name=f"q{j}")
                rxt = mid_pool.tile([P, tw], bf16, tag="rxt", name=f"rxt{j}")
                ja = jnk_pool.tile([P, tw], bf16, tag="ja", name=f"ja{j}")
                jd = jnk_pool.tile([P, tw], bf16, tag="jd", name=f"jd{j}")

                # --- DVE: fast-mode ops only, no accumulators ---
                nc.vector.tensor_add(out=H, in0=lo, in1=up)
                nc.vector.tensor_sub(out=w, in0=lo, in1=up)
                nc.vector.tensor_scalar(
                    out=c, in0=H, scalar1=0.5, scalar2=None, op0=Alu.mult
                )
                nc.vector.tensor_sub(out=x, in0=c, in1=p_t)
                nc.vector.tensor_sub(out=e, in0=c, in1=t_t)
                nc.vector.tensor_mul(out=q, in0=v_t, in1=x)
                nc.vector.tensor_scalar(
                    out=rxt, in0=x, scalar1=0.0, scalar2=None, op0=Alu.max
                )

                # --- ACT: the two nonlinear accumulations ---
                nc.scalar.activation(
                    out=jd, in_=w, func=Act.Relu,
                    accum_out=acc_act[:, n_tiles + j : n_tiles + j + 1],
                )
                nc.scalar.activation(
                    out=ja, in_=e, func=Act.Square,
                    accum_out=acc_act[:, j : j + 1],
                )

                # --- PE: column-sum matmuls (single ones stationary) ---
                for ci, ch in enumerate(range(0, tw, MM)):
                    is_last = done_chunks + ci == n_chunks - 1
                    nc.tensor.matmul(
                        ps_w, onesv, w[:, ch : ch + MM],
                        start=first[0], stop=is_last,
                    )
                    first[0] = False
                for ci, ch in enumerate(range(0, tw, MM)):
                    is_last = done_chunks + ci == n_chunks - 1
                    nc.tensor.matmul(
                        ps_rx, onesv, rxt[:, ch : ch + MM],
                        start=first[1], stop=is_last,
                    )
                    first[1] = False
                for ci, ch in enumerate(range(0, tw, MM)):
                    is_last = done_chunks + ci == n_chunks - 1
                    nc.tensor.matmul(
                        ps_vx, onesv, q[:, ch : ch + MM],
                        start=first[2], stop=is_last,
                    )
                    first[2] = False
                done_chunks += tw // MM

            # PSUM -> SBUF -> DRAM for the column sums
            nc.scalar.activation(out=sw_sb[:, :], in_=ps_w, func=Act.Copy)
            nc.scalar.activation(out=srx_sb[:, :], in_=ps_rx, func=Act.Copy)
            nc.scalar.activation(out=svx_sb[:, :], in_=ps_vx, func=Act.Copy)

            nc.sync.dma_start(out=out[:, :], in_=acc_act)
            nc.sync.dma_start(out=sums[0:1, :], in_=sw_sb)
            nc.sync.dma_start(out=sums[1:2, :], in_=srx_sb)
            nc.sync.dma_start(out=sums[2:3, :], in_=svx_sb)

    nc.compile()
    return nc


def _get_nc():
    key = (FPL, TILE_WIDTHS)
    if key not in _NC_CACHE:
        _NC_CACHE[key] = _build()
    return _NC_CACHE[key]


def _make_consts():
    import ml_dtypes

    return np.ones((P, 1), dtype=ml_dtypes.bfloat16)


def _shard(inputs, fpl=FPL, widths=TILE_WIDTHS, n_cores=N_CORES):
    import ml_dtypes

    bf = ml_dtypes.bfloat16
    n = n_cores * P * fpl
    pred = np.asarray(inputs["pred"])
    targ = np.asarray(inputs["target"]).reshape(n)
    prev = np.asarray(inputs["prev_pci"]).reshape(n)
    # int64 is unsupported on-device; values are 0/1 so a bf16 cast is exact.
    pv = np.asarray(inputs["pv_values"]).astype(bf).reshape(n)

    lo = pred[:, 0].astype(bf)
    up = pred[:, 1].astype(bf)
    tb = targ.astype(bf)
    pb = prev.astype(bf)

    consts = _make_consts()
    np_per_core = P * fpl

    in_maps = []
    for cix in range(n_cores):
        s = slice(cix * np_per_core, (cix + 1) * np_per_core)
        streams = (
            lo[s].reshape(P, fpl),
            up[s].reshape(P, fpl),
            tb[s].reshape(P, fpl),
            pb[s].reshape(P, fpl),
            pv[s].reshape(P, fpl),
        )
        # tile-major: per partition, each tile's 5 stream-chunks contiguous
        parts = []
        off = 0
        for fd in widths:
            for st in streams:
                parts.append(st[:, off : off + fd])
            off += fd
        big = np.concatenate(parts, axis=1)
        in_maps.append({"big": np.ascontiguousarray(big), "consts": consts})
    return in_maps


def _combine(core_outs, core_sums, widths=TILE_WIDTHS, n=N):
    """core_outs: [P, 2*n_tiles] ACT accumulators per core (S_sq | S_vd).
    core_sums: [3, MM] column sums per core (S_w | S_rx | S_vx)."""
    n_tiles = len(widths)
    s_sq = s_vd = s_w = s_rx = s_vx = 0.0
    for o, ss in zip(core_outs, core_sums):
        o = np.asarray(o, dtype=np.float64)
        ss = np.asarray(ss, dtype=np.float64)
        s_sq += o[:, 0:n_tiles].sum()
        s_vd += o[:, n_tiles : 2 * n_tiles].sum()
        s_w += ss[0].sum()
        s_rx += ss[1].sum()
        s_vx += ss[2].sum()

    center_loss = s_sq / n
    width_loss = -s_w / n                  # sum(up - lo) = -sum(lo - up)
    valid_penalty = s_vd / n
    direction_penalty = s_rx - s_vx
    total = (
        center_loss * 10.0
        + 0.1 * width_loss
        + 10.0 * valid_penalty
        + 0.5 * direction_penalty / n
    )
    return np.float32(total)


def _run(inputs, trace=False):
    """Run the SPMD kernel; returns (scalar_result, BassKernelResults)."""
    from concourse.bass_utils import run_bass_kernel_spmd

    nc = _get_nc()
    in_maps = _shard(inputs)
    res = run_bass_kernel_spmd(
        nc, in_maps, core_ids=list(range(N_CORES)), trace=trace
    )
    core_outs = [res.results[c]["out"] for c in range(N_CORES)]
    core_sums = [res.results[c]["sums"] for c in range(N_CORES)]
    return _combine(core_outs, core_sums), res


def kernel(**inputs) -> np.ndarray:
    result, _ = _run(inputs, trace=False)
    return result


# revision 44
# speedup vs baseline: 1.0852x; 1.0692x over previous
"""Trainium2 Bass kernel for the interval-prediction custom loss.

total = 10*mean((t - (l+u)/2)^2) + 0.1*mean(u-l) + 10*mean(relu(l-u))
        + 0.5*sum(where(pv==0, relu(c-p), relu(p-c)))/N        with c=(l+u)/2

Strategy: pure data parallel over N across 8 NeuronCores; host does only the
tiny final scalar reduction in float64.

Engine plan (v13).  Measured facts from earlier traces: DVE tensor_tensor
runs 2x and tensor_scalar 4x at bf16, but ANY DVE op with an accumulator
drops to 1x; GPSIMD elementwise ops steal the DVE's SBUF port; heavy PE
matmul traffic slows the DMA streams; ACT costs ~(fd+352)/1.2 ns per pass
regardless of dtype and applies a free affine (scale) before its function.

The host re-encodes t and p as t2 = 2t, p2 = 2p (lossless bf16 exponent
shift), which removes the c = 0.5*(lo+up) pass entirely:

    x2 = H - p2 = 2*(c - p),  e2 = H - t2 = 2*(c - t)
    sum((c-t)^2) = sum(Square(0.5 * e2))      (ACT free affine)
    relu/product sums of x2 are halved on the host.

  DVE:    H  = lo + up           (TT 2x)
          w  = lo - up           (TT 2x)
          e2 = H - t2            (TT 2x)   [needs only dma_a]
          x2 = H - p2            (TT 2x)
          q  = v * x2            (TT 2x)
          rxt = max(x2, 0)       (TS 4x)
  ACT:    Relu(w)             accum -> S_vd  (= sum relu(lo-up))
          Square(e2, s=0.5)   accum -> S_sq
  PE:     ones^T * w   -> PSUM[1,512]   (S_w;  width sum = -S_w)
          ones^T * rxt -> PSUM[1,512]   (2*S_rx)
          ones^T * q   -> PSUM[1,512]   (2*S_vx)

Host combine: width = -S_w/N, direction = (S_rx2 - S_vx2)/2.

All five streams are bf16 (int64 pv is 0/1 so the cast is exact); all
accumulation is fp32.  Tile widths are multiples of 512 so each PE matmul
chunk is exactly 512 (one PSUM bank).  dma_a carries [lo,up,t2] (feeds the
first four DVE ops and both ACT passes), dma_b carries [p2,v]; io pools are
deep enough that no dma_start ever waits on a tile free (head-of-line
blocking on the sync sequencer stalls all 16 DMA queues).
"""

import sys

if "/opt/trn_rl_repo" not in sys.path:
    sys.path.insert(0, "/opt/trn_rl_repo")

import numpy as np

N = 8388608
N_CORES = 8
P = 128
NP_PER_CORE = N // N_CORES            # 1048576
FPL = NP_PER_CORE // P                # 8192 elements per partition lane
TILE_WIDTHS = (512, 2048, 2048, 2048, 1024, 512)
assert sum(TILE_WIDTHS) == FPL
assert all(w % 512 == 0 for w in TILE_WIDTHS)
MM = 512                              # matmul moving chunk / PSUM bank width

_NC_CACHE = {}


def _build(fpl=FPL, widths=TILE_WIDTHS):
    """Build the per-core Bass program (identical on all cores)."""
    from concourse import bacc, mybir
    from concourse.tile import TileContext

    assert sum(widths) == fpl
    n_tiles = len(widths)

    f32 = mybir.dt.float32
    bf16 = mybir.dt.bfloat16
    Alu = mybir.AluOpType
    Act = mybir.ActivationFunctionType

    nc = bacc.Bacc(trn_type="TRN2")
    big = nc.declare_dram_parameter("big", [P, 5 * fpl], bf16, isOutput=False)
    consts = nc.declare_dram_parameter("consts", [P, 1], bf16, isOutput=False)
    # accumulator columns: S_sq (n_tiles) | S_vd (n_tiles)
    out = nc.declare_dram_parameter("out", [P, 2 * n_tiles], f32, isOutput=True)
    # rows: S_w | 2*S_rx | 2*S_vx column sums
    sums = nc.declare_dram_parameter("sums", [3, MM], f32, isOutput=True)

    with TileContext(nc) as tc:
        with (
            tc.tile_pool(name="ioa", bufs=5) as ioa_pool,
            tc.tile_pool(name="iob", bufs=5) as iob_pool,
            tc.tile_pool(name="mid", bufs=3) as mid_pool,
            tc.tile_pool(name="jnk", bufs=2) as jnk_pool,
            tc.tile_pool(name="acc", bufs=1) as acc_pool,
            tc.tile_pool(name="pss", bufs=1, space="PSUM") as pss_pool,
        ):
            acc_act = acc_pool.tile([P, 2 * n_tiles], f32, tag="acc_act")
            sw_sb = acc_pool.tile([1, MM], f32, tag="sw_sb")
            srx_sb = acc_pool.tile([1, MM], f32, tag="srx_sb")
            svx_sb = acc_pool.tile([1, MM], f32, tag="svx_sb")
            ps_w = pss_pool.tile([1, MM], f32, tag="ps_w")
            ps_rx = pss_pool.tile([1, MM], f32, tag="ps_rx")
            ps_vx = pss_pool.tile([1, MM], f32, tag="ps_vx")

            const_t = acc_pool.tile([P, 1], bf16, tag="consts")
            onesv = const_t[:, 0:1]

            off = 0
            first = [True, True, True]
            n_chunks = fpl // MM
            done_chunks = 0
            for j, tw in enumerate(widths):
                big_a = ioa_pool.tile([P, 3, tw], bf16, tag="biga", name=f"biga{j}")
                big_b = iob_pool.tile([P, 2, tw], bf16, tag="bigb", name=f"bigb{j}")
                src = big[:, off : off + 5 * tw].rearrange("p (s f) -> p s f", s=5)
                nc.sync.dma_start(out=big_a, in_=src[:, 0:3, :])
                if j == 0:
                    # consts needed only by the PE; issue after the first
                    # compute-critical DMA
                    nc.sync.dma_start(out=const_t, in_=consts[:, :])
                nc.sync.dma_start(out=big_b, in_=src[:, 3:5, :])
                off += 5 * tw

                lo = big_a[:, 0, :]
                up = big_a[:, 1, :]
                t_t = big_a[:, 2, :]
                p_t = big_b[:, 0, :]
                v_t = big_b[:, 1, :]

                H = mid_pool.tile([P, tw], bf16, tag="H", name=f"H{j}")
                w = mid_pool.tile([P, tw], bf16, tag="w", name=f"w{j}")
                e = mid_pool.tile([P, tw], bf16, tag="e", name=f"e{j}")
                x = mid_pool.tile([P, tw], bf16, tag="x", name=f"x{j}")
                q = mid_pool.tile([P, tw], bf16, tag="q", name=f"q{j}")
                rxt = mid_pool.tile([P, tw], bf16, tag="rxt", name=f"rxt{j}")
                ja = jnk_pool.tile([P, tw], bf16, tag="ja", name=f"ja{j}")
                jd = jnk_pool.tile([P, tw], bf16, tag="jd", name=f"jd{j}")

                # --- DVE: fast-mode ops only, no accumulators ---
                nc.vector.tensor_add(out=H, in0=lo, in1=up)
                nc.vector.tensor_sub(out=w, in0=lo, in1=up)
                nc.vector.tensor_sub(out=e, in0=H, in1=t_t)
                nc.vector.tensor_sub(out=x, in0=H, in1=p_t)
                nc.vector.tensor_mul(out=q, in0=v_t, in1=x)
                nc.vector.tensor_scalar(
                    out=rxt, in0=x, scalar1=0.0, scalar2=None, op0=Alu.max
                )

                # --- ACT: the two nonlinear accumulations ---
                nc.scalar.activation(
                    out=jd, in_=w, func=Act.Relu,
                    accum_out=acc_act[:, n_tiles + j : n_tiles + j + 1],
                )
                nc.scalar.activation(
                    out=ja, in_=e, func=Act.Square, scale=0.5,
                    accum_out=acc_act[:, j : j + 1],
                )

                # --- PE: column-sum matmuls (single ones stationary) ---
                for ci, ch in enumerate(range(0, tw, MM)):
                    is_last = done_chunks + ci == n_chunks - 1
                    nc.tensor.matmul(
                        ps_w, onesv, w[:, ch : ch + MM],
                        start=first[0], stop=is_last,
                    )
                    first[0] = False
                for ci, ch in enumerate(range(0, tw, MM)):
                    is_last = done_chunks + ci == n_chunks - 1
                    nc.tensor.matmul(
                        ps_rx, onesv, rxt[:, ch : ch + MM],
                        start=first[1], stop=is_last,
                    )
                    first[1] = False
                for ci, ch in enumerate(range(0, tw, MM)):
                    is_last = done_chunks + ci == n_chunks - 1
                    nc.tensor.matmul(
                        ps_vx, onesv, q[:, ch : ch + MM],
                        start=first[2], stop=is_last,
                    )
                    first[2] = False
                done_chunks += tw // MM

            # PSUM -> SBUF -> DRAM for the column sums
            nc.scalar.activation(out=sw_sb[:, :], in_=ps_w, func=Act.Copy)
            nc.scalar.activation(out=srx_sb[:, :], in_=ps_rx, func=Act.Copy)
            nc.scalar.activation(out=svx_sb[:, :], in_=ps_vx, func=Act.Copy)

            nc.sync.dma_start(out=out[:, :], in_=acc_act)
            nc.sync.dma_start(out=sums[0:1, :], in_=sw_sb)
            nc.sync.dma_start(out=sums[1:2, :], in_=srx_sb)
            nc.sync.dma_start(out=sums[2:3, :], in_=svx_sb)

    nc.compile()
    return nc


def _get_nc():
    key = (FPL, TILE_WIDTHS)
    if key not in _NC_CACHE:
        _NC_CACHE[key] = _build()
    return _NC_CACHE[key]


def _make_consts():
    import ml_dtypes

    return np.ones((P, 1), dtype=ml_dtypes.bfloat16)


def _shard(inputs, fpl=FPL, widths=TILE_WIDTHS, n_cores=N_CORES):
    import ml_dtypes

    bf = ml_dtypes.bfloat16
    n = n_cores * P * fpl
    pred = np.asarray(inputs["pred"])
    targ = np.asarray(inputs["target"]).reshape(n)
    prev = np.asarray(inputs["prev_pci"]).reshape(n)
    # int64 is unsupported on-device; values are 0/1 so a bf16 cast is exact.
    pv = np.asarray(inputs["pv_values"]).astype(bf).reshape(n)

    lo = pred[:, 0].astype(bf)
    up = pred[:, 1].astype(bf)
    # lossless bf16 re-encoding: 2*t and 2*p are exponent shifts, which lets
    # the kernel skip the 0.5*(lo+up) pass (see module docstring)
    tb = (2.0 * targ).astype(bf)
    pb = (2.0 * prev).astype(bf)

    consts = _make_consts()
    np_per_core = P * fpl

    in_maps = []
    for cix in range(n_cores):
        s = slice(cix * np_per_core, (cix + 1) * np_per_core)
        streams = (
            lo[s].reshape(P, fpl),
            up[s].reshape(P, fpl),
            tb[s].reshape(P, fpl),
            pb[s].reshape(P, fpl),
            pv[s].reshape(P, fpl),
        )
        # tile-major: per partition, each tile's 5 stream-chunks contiguous
        parts = []
        off = 0
        for fd in widths:
            for st in streams:
                parts.append(st[:, off : off + fd])
            off += fd
        big = np.concatenate(parts, axis=1)
        in_maps.append({"big": np.ascontiguousarray(big), "consts": consts})
    return in_maps


def _combine(core_outs, core_sums, widths=TILE_WIDTHS, n=N):
    """core_outs: [P, 2*n_tiles] ACT accumulators per core (S_sq | S_vd).
    core_sums: [3, MM] column sums per core (S_w | 2*S_rx | 2*S_vx)."""
    n_tiles = len(widths)
    s_sq = s_vd = s_w = s_rx2 = s_vx2 = 0.0
    for o, ss in zip(core_outs, core_sums):
        o = np.asarray(o, dtype=np.float64)
        ss = np.asarray(ss, dtype=np.float64)
        s_sq += o[:, 0:n_tiles].sum()
        s_vd += o[:, n_tiles : 2 * n_tiles].sum()
        s_w += ss[0].sum()
        s_rx2 += ss[1].sum()
        s_vx2 += ss[2].sum()

    center_loss = s_sq / n
    width_loss = -s_w / n                  # sum(up - lo) = -sum(lo - up)
    valid_penalty = s_vd / n
    direction_penalty = (s_rx2 - s_vx2) / 2.0
    total = (
        center_loss * 10.0
        + 0.1 * width_loss
        + 10.0 * valid_penalty
        + 0.5 * direction_penalty / n
    )
    return np.float32(total)


def _run(inputs, trace=False):
    """Run the SPMD kernel; returns (scalar_result, BassKernelResults)."""
    from concourse.bass_utils import run_bass_kernel_spmd

    nc = _get_nc()
    in_maps = _shard(inputs)
    res = run_bass_kernel_spmd(
        nc, in_maps, core_ids=list(range(N_CORES)), trace=trace
    )
    core_outs = [res.results[c]["out"] for c in range(N_CORES)]
    core_sums = [res.results[c]["sums"] for c in range(N_CORES)]
    return _combine(core_outs, core_sums), res


def kernel(**inputs) -> np.ndarray:
    result, _ = _run(inputs, trace=False)
    return result
